# revision 4
# baseline (speedup 1.0000x reference)
"""Trainium2 Bass kernel for nn_CrossAttention (B=4, N=M=2048, DIM=1024, H=16, Dh=64).

Wire-optimized design (the axon tunnel moves ~75 MB/s, so transfer bytes are the
metric that matters):
  - 4 cores, one batch each, all 16 heads per core -> x/context are sent exactly
    once (no duplication), y comes back complete (no host partial sums).
  - x/context cross the wire in bf16, transposed on host; y returns in bf16.
  - Weights are uploaded once and kept device-resident across kernel() calls
    (verified by array_equal on repeat calls); output zero-buffers stay resident.
  - The jitted SPMD callable is built once and cached at module level.

Device program (per core, 16 heads, bf16 matmuls for QKV/S/PV, f32r for the
output projection):
  xT, cT:  [1024, 2048] bf16 (contraction dim on partitions)
  Q^T/K^T: [1024, 2048]  (inner on partitions) -> QK^T contraction over d=64
  S^T:     [m, n] tiles  (keys on partitions)  -> context-mask bias is
           per-partition, folded into the ACT Exp (bias + scale=1/8)
  V':      [m, 65*16]    V plus a ones-column per head -> PV matmul also emits
           softmax denominators (row 64 of the [65, n] psum).
  Normalization: 1/s premultiplied by x_mask (zeroes masked query rows on
  device), broadcast via selector matmul, one DVE multiply on O^T.
"""

import sys
import numpy as np

sys.path.insert(0, "/opt/trn_rl_repo")

import concourse.bass as bass  # noqa: E402
import concourse.tile as tile  # noqa: E402
from concourse import mybir  # noqa: E402
from contextlib import ExitStack  # noqa: E402

F32 = mybir.dt.float32
F32R = mybir.dt.float32r
BF16 = mybir.dt.bfloat16
NP_BF16 = mybir.dt.np(BF16)
EXP = mybir.ActivationFunctionType.Exp
MULT = mybir.AluOpType.mult

B, N, M, DIM = 4, 2048, 2048, 1024
HEADS, DH = 16, 64
N_CORES = 4
MASK_BIAS = -10000.0


def _legalize_waits(nc):
    """This walrus build accepts at most one sync-wait per TPB instruction;
    hoist extra waits onto single-wait NoOps on the same engine queue."""
    ctr = 0

    def fix(bb):
        nonlocal ctr
        new_insts, changed = [], False
        for inst in bb.instructions:
            si = inst.sync_info
            if si is not None and si.on_wait is not None and len(si.on_wait) > 1:
                waits = list(si.on_wait)
                for w in waits[:-1]:
                    ctr += 1
                    new_insts.append(mybir.InstNoOp(
                        name=f"waitnop-{ctr}", engine=inst.engine, ins=[], outs=[],
                        sync_info=mybir.SyncInfo(on_wait=[w], on_update=[]),
                    ))
                inst.sync_info = mybir.SyncInfo(
                    on_wait=[waits[-1]], on_update=list(si.on_update or []))
                changed = True
            new_insts.append(inst)
        if changed:
            bb.instructions.clear()
            for i in new_insts:
                bb.add_instruction(i)

    for fn in nc.m.functions:
        for bb in fn.blocks:
            fix(bb)
    for q in nc.m.queues or []:
        for bb in q.blocks:
            fix(bb)
    return ctr


def build_program():
    nc = bass.Bass()
    # per-call inputs
    xT_d = nc.dram_tensor("xT", [DIM, N], BF16, kind="ExternalInput")
    cT_d = nc.dram_tensor("cT", [DIM, M], BF16, kind="ExternalInput")
    bias_d = nc.dram_tensor("bias", [128, 16], F32, kind="ExternalInput")
    xm_d = nc.dram_tensor("xm", [HEADS, N], F32, kind="ExternalInput")
    # resident inputs (weights/constants)
    wq_d = nc.dram_tensor("wq", [DIM, DIM], BF16, kind="ExternalInput")
    wk_d = nc.dram_tensor("wk", [DIM, DIM], BF16, kind="ExternalInput")
    wv_d = nc.dram_tensor("wv", [DIM, DIM], BF16, kind="ExternalInput")
    wo_d = nc.dram_tensor("wo", [DIM, DIM], F32R, kind="ExternalInput")
    sel_d = nc.dram_tensor("sel", [HEADS, DIM], F32R, kind="ExternalInput")
    ones_d = nc.dram_tensor("ones", [128, 16], BF16, kind="ExternalInput")
    y_d = nc.dram_tensor("y", [N, DIM], BF16, kind="ExternalOutput")
    oscr_d = nc.dram_tensor("oscr", [8, 128, N], F32R)  # internal scratch

    KT = DIM // 128  # 8 contraction tiles
    with tile.TileContext(nc) as tc, ExitStack() as ctx:
        persist = ctx.enter_context(tc.tile_pool(name="persist", bufs=1))
        psum = ctx.enter_context(tc.tile_pool(name="psum", bufs=2, space="PSUM"))
        psumO = ctx.enter_context(tc.tile_pool(name="psumO", bufs=4, space="PSUM"))

        bias_sb = persist.tile([128, 16], F32, name="bias_sb")
        sel_sb = persist.tile([HEADS, DIM], F32R, name="sel_sb")
        xm_sb = persist.tile([HEADS, N], F32, name="xm_sb")
        s_sb = persist.tile([HEADS, N], F32, name="s_sb")

        nc.sync.dma_start(out=bias_sb, in_=bias_d[:, :])
        nc.sync.dma_start(out=sel_sb, in_=sel_d[:, :])
        nc.sync.dma_start(out=xm_sb, in_=xm_d[:, :])

        # pool spanning phases A+B (K^T and V' persist through attention)
        ctxAB = ctx.enter_context(ExitStack())
        ab = ctxAB.enter_context(tc.tile_pool(name="ab", bufs=1))
        kT = [ab.tile([128, M], BF16, name=f"kT{pt}") for pt in range(KT)]
        vv = [ab.tile([128, 65 * HEADS], BF16, name=f"vv{mt}") for mt in range(16)]

        # ---------------- Phase A: K^T and V' projections -------------------
        with tc.tile_pool(name="phaseA", bufs=1) as pa:
            cT = [pa.tile([128, M], BF16, name=f"cT{kt}") for kt in range(KT)]
            wk = [pa.tile([128, DIM], BF16, name=f"wk{kt}") for kt in range(KT)]
            wv = [pa.tile([128, DIM], BF16, name=f"wv{kt}") for kt in range(KT)]
            cT_t = cT_d.rearrange("(ko p) m -> ko p m", p=128)
            wk_t = wk_d.rearrange("(ko p) c -> ko p c", p=128)
            wv_t = wv_d.rearrange("(ko p) c -> ko p c", p=128)
            for kt in range(KT):
                nc.sync.dma_start(out=cT[kt], in_=cT_t[kt])
                nc.sync.dma_start(out=wk[kt], in_=wk_t[kt])
                nc.sync.dma_start(out=wv[kt], in_=wv_t[kt])

            # K^T: [1024 inner, 2048 m]
            for pt in range(KT):
                for t in range(2):
                    ps = psum.tile([128, 1024], F32, name="ps", tag="ps")
                    for kt in range(KT):
                        for sl in range(2):
                            nc.tensor.matmul(
                                ps[:, sl * 512:(sl + 1) * 512],
                                wk[kt][:, pt * 128:(pt + 1) * 128],
                                cT[kt][:, (2 * t + sl) * 512:(2 * t + sl + 1) * 512],
                                start=(kt == 0), stop=(kt == KT - 1))
                    nc.vector.tensor_copy(
                        out=kT[pt][:, t * 1024:(t + 1) * 1024], in_=ps)

            # V': [m, 65 per head] with ones column at 65j+64
            for mt in range(16):
                vvv = vv[mt].rearrange("p (j c) -> p j c", c=65)
                nc.sync.dma_start(out=vvv[:, :, 64], in_=ones_d[:, :])
            for mt in range(16):
                ps = psum.tile([128, 1024], F32, name="ps", tag="ps")
                for sl in range(2):
                    for kt in range(KT):
                        nc.tensor.matmul(
                            ps[:, sl * 512:(sl + 1) * 512],
                            cT[kt][:, mt * 128:(mt + 1) * 128],
                            wv[kt][:, sl * 512:(sl + 1) * 512],
                            start=(kt == 0), stop=(kt == KT - 1))
                for j in range(HEADS):
                    nc.vector.tensor_copy(
                        out=vv[mt][:, 65 * j:65 * j + 64],
                        in_=ps[:, 64 * j: 64 * j + 64])

        # ---------------- Phase B: per head-pair attention -------------------
        ctxB = ctx.enter_context(ExitStack())
        pb = ctxB.enter_context(tc.tile_pool(name="phaseB", bufs=1))
        xT = [pb.tile([128, N], BF16, name=f"xT{kt}") for kt in range(KT)]
        xT_t = xT_d.rearrange("(ko p) n -> ko p n", p=128)
        for kt in range(KT):
            nc.sync.dma_start(out=xT[kt], in_=xT_t[kt])

        wqp_pool = ctxB.enter_context(tc.tile_pool(name="wqp", bufs=2))
        qt_pool = ctxB.enter_context(tc.tile_pool(name="qt", bufs=1))
        pt_pool = ctxB.enter_context(tc.tile_pool(name="ptp", bufs=3))
        st_pool = ctxB.enter_context(tc.tile_pool(name="stp", bufs=4))
        ot_pool = ctxB.enter_context(tc.tile_pool(name="otp", bufs=2))

        for p in range(8):
            wqp = wqp_pool.tile([128, KT, 128], BF16, name="wqp", tag="wqp")
            for kt in range(KT):
                nc.sync.dma_start(
                    out=wqp[:, kt, :],
                    in_=wq_d[kt * 128:(kt + 1) * 128, p * 128:(p + 1) * 128])

            # Q^T for this pair: [128 inner, 2048 n]
            qT = qt_pool.tile([128, N], BF16, name="qT", tag="qT")
            for t in range(2):
                ps = psum.tile([128, 1024], F32, name="ps", tag="ps")
                for kt in range(KT):
                    for sl in range(2):
                        nc.tensor.matmul(
                            ps[:, sl * 512:(sl + 1) * 512],
                            wqp[:, kt, :],
                            xT[kt][:, (2 * t + sl) * 512:(2 * t + sl + 1) * 512],
                            start=(kt == 0), stop=(kt == KT - 1))
                nc.vector.tensor_copy(out=qT[:, t * 1024:(t + 1) * 1024], in_=ps)

            oT_p = ot_pool.tile([128, N], F32R, name="oT_p", tag="oT_p")
            for nt2 in range(2):
                psO = [psumO.tile([65, 512], F32, name="psO", tag="psO")
                       for _ in range(4)]
                for mt in range(16):
                    for side in range(2):
                        rows = slice(side * 64, side * 64 + 64)
                        jj = 2 * p + side
                        psS = psum.tile([128, 1024], F32, name="ps", tag="ps")
                        for ncs in range(2):
                            nt_c = nt2 * 1024 + ncs * 512
                            nc.tensor.matmul(
                                psS[:, ncs * 512:(ncs + 1) * 512],
                                kT[p][rows, mt * 128:(mt + 1) * 128],
                                qT[rows, nt_c:nt_c + 512],
                                start=True, stop=True,
                                tile_position=(side * 64, 0))
                        pt_t = pt_pool.tile([128, 1024], BF16, name="pt_t", tag="pt")
                        nc.scalar.activation(
                            out=pt_t, in_=psS, func=EXP,
                            bias=bias_sb[:, mt:mt + 1], scale=0.125)
                        for ncs in range(2):
                            nc.tensor.matmul(
                                psO[side * 2 + ncs],
                                vv[mt][:, 65 * jj:65 * jj + 65],
                                pt_t[:, ncs * 512:(ncs + 1) * 512],
                                start=(mt == 0), stop=(mt == 15))
                for side in range(2):
                    jj = 2 * p + side
                    for ncs in range(2):
                        po = psO[side * 2 + ncs]
                        c0 = nt2 * 1024 + ncs * 512
                        chunk = slice(c0, c0 + 512)
                        if side == 0:
                            nc.vector.tensor_copy(out=oT_p[0:64, chunk], in_=po[0:64, :])
                            st = st_pool.tile([65, 512], F32R, name="st", tag="st")
                            nc.vector.tensor_copy(out=st[64:65, :], in_=po[64:65, :])
                            nc.sync.dma_start(out=s_sb[jj:jj + 1, chunk], in_=st[64:65, :].bitcast(F32))
                        else:
                            st = st_pool.tile([65, 512], F32R, name="st", tag="st")
                            nc.vector.tensor_copy(out=st, in_=po)
                            nc.sync.dma_start(out=oT_p[64:128, chunk], in_=st[0:64, :])
                            nc.sync.dma_start(out=s_sb[jj:jj + 1, chunk], in_=st[64:65, :].bitcast(F32))
            nc.sync.dma_start(out=oscr_d[p], in_=oT_p)
        ctxB.close()
        ctxAB.close()

        # ---------------- Phase C: normalize + output projection -------------
        with tc.tile_pool(name="phaseC", bufs=1) as pc, \
             tc.tile_pool(name="ypool", bufs=2) as ypool:
            oTc = [pc.tile([128, N], F32R, name=f"oTc{pt}") for pt in range(KT)]
            wo_sb = [pc.tile([128, DIM], F32R, name=f"wo{kt}") for kt in range(KT)]
            recip_f = pc.tile([HEADS, N], F32, name="recip_f")
            recip_m = pc.tile([HEADS, N], F32, name="recip_m")
            recip_r = pc.tile([HEADS, N], F32R, name="recip_r")
            for pt in range(KT):
                nc.sync.dma_start(out=oTc[pt], in_=oscr_d[pt])
            wo_t = wo_d.rearrange("(ko p) c -> ko p c", p=128)
            for kt in range(KT):
                nc.sync.dma_start(out=wo_sb[kt], in_=wo_t[kt])
            nc.vector.reciprocal(out=recip_f, in_=s_sb)
            nc.vector.tensor_tensor(out=recip_m, in0=recip_f, in1=xm_sb, op=MULT)
            nc.vector.tensor_copy(out=recip_r, in_=recip_m)

            for pt in range(KT):
                for ncr in range(2):
                    psR = psum.tile([128, 1024], F32, name="ps", tag="ps")
                    for sl in range(2):
                        c0 = (ncr * 2 + sl) * 512
                        nc.tensor.matmul(
                            psR[:, sl * 512:(sl + 1) * 512],
                            sel_sb[:, pt * 128:(pt + 1) * 128],
                            recip_r[:, c0:c0 + 512],
                            start=True, stop=True)
                    nc.vector.tensor_tensor(
                        out=oTc[pt][:, ncr * 1024:(ncr + 1) * 1024],
                        in0=oTc[pt][:, ncr * 1024:(ncr + 1) * 1024],
                        in1=psR, op=MULT)

            for nt in range(16):
                psY = psum.tile([128, 1024], F32, name="ps", tag="ps")
                for half in range(2):
                    for kt in range(KT):
                        nc.tensor.matmul(
                            psY[:, half * 512:(half + 1) * 512],
                            oTc[kt][:, nt * 128:(nt + 1) * 128],
                            wo_sb[kt][:, half * 512:(half + 1) * 512],
                            start=(kt == 0), stop=(kt == KT - 1))
                y_t = ypool.tile([128, DIM], BF16, name="y_t", tag="y_t")
                nc.vector.tensor_copy(out=y_t, in_=psY)
                nc.sync.dma_start(out=y_d[nt * 128:(nt + 1) * 128, :], in_=y_t)

    _legalize_waits(nc)
    return nc


_STATE = {}


def get_program():
    if "nc" not in _STATE:
        _STATE["nc"] = build_program()
    return _STATE["nc"]


def _build_executor():
    import jax
    from jax.sharding import Mesh, PartitionSpec, NamedSharding
    from jax.experimental.shard_map import shard_map
    from concourse import bass2jax

    bass2jax.install_neuronx_cc_hook()
    nc = get_program()

    partition_name = nc.partition_id_tensor.name if nc.partition_id_tensor else None
    in_names, out_names, out_avals, zero_outs = [], [], [], []
    for alloc in nc.m.functions[0].allocations:
        if not isinstance(alloc, mybir.MemoryLocationSet):
            continue
        name = alloc.memorylocations[0].name
        if alloc.kind == "ExternalInput":
            if name != partition_name:
                in_names.append(name)
        elif alloc.kind == "ExternalOutput":
            out_names.append(name)
            shape = tuple(alloc.tensor_shape)
            dtype = mybir.dt.np(alloc.dtype)
            out_avals.append(jax.core.ShapedArray(shape, dtype))
            zero_outs.append(np.zeros(shape, dtype))
    n_params = len(in_names)
    n_outs = len(out_avals)
    all_in_names = list(in_names) + list(out_names)
    if partition_name is not None:
        all_in_names.append(partition_name)

    def _body(*args):
        ins = args[:n_params]
        zeros = list(args[n_params:])
        operands = list(ins) + zeros
        if partition_name is not None:
            operands.append(bass2jax.partition_id_tensor())
        outs = bass2jax._bass_exec_p.bind(
            *operands,
            out_avals=tuple(out_avals),
            in_names=tuple(all_in_names),
            out_names=tuple(out_names),
            lowering_input_output_aliases=(),
            sim_require_finite=False,
            sim_require_nnan=False,
            nc=nc,
        )
        return tuple(outs)

    devices = jax.devices()[:N_CORES]
    mesh = Mesh(np.asarray(devices), ("core",))
    sharded = jax.jit(
        shard_map(_body, mesh=mesh,
                  in_specs=(PartitionSpec("core"),) * (n_params + n_outs),
                  out_specs=(PartitionSpec("core"),) * n_outs,
                  check_rep=False),
        keep_unused=True)
    sharding = NamedSharding(mesh, PartitionSpec("core"))

    dev_zeros = [jax.device_put(
        np.zeros((N_CORES * z.shape[0], *z.shape[1:]), z.dtype), sharding)
        for z in zero_outs]
    jax.block_until_ready(dev_zeros)

    st = {"jax": jax, "sharding": sharding, "sharded": sharded,
          "in_names": in_names, "out_names": out_names,
          "out_avals": out_avals, "dev_zeros": dev_zeros,
          "res_names": ("wq", "wk", "wv", "wo", "sel", "ones"),
          "res_dev": None, "res_src": None}
    return st


def get_executor():
    if "exec" not in _STATE:
        _STATE["exec"] = _build_executor()
    return _STATE["exec"]


def _resident_weights(st, Wq, Wkv, Wo):
    """Upload weights/constants once; reuse device copies while they match."""
    jax = st["jax"]
    src = (Wq, Wkv, Wo)
    if st["res_dev"] is not None:
        old = st["res_src"]
        same = all(o is n for o, n in zip(old, src))
        if not same:
            same = all(np.array_equal(o, n) for o, n in zip(old, src))
        if same:
            return st["res_dev"]
    sel = np.zeros((HEADS, DIM), np.float32)
    for j in range(HEADS):
        sel[j, 64 * j:64 * j + 64] = 1.0
    host = {
        "wq": Wq.astype(NP_BF16),
        "wk": Wkv[:, :DIM].astype(NP_BF16),
        "wv": Wkv[:, DIM:].astype(NP_BF16),
        "wo": Wo.astype(np.float32),
        "sel": sel,
        "ones": np.ones((128, 16), NP_BF16),
    }
    dev = {}
    for name, arr in host.items():
        tiled = np.ascontiguousarray(
            np.broadcast_to(arr[None], (N_CORES, *arr.shape))
        ).reshape(N_CORES * arr.shape[0], *arr.shape[1:])
        dev[name] = jax.device_put(tiled, st["sharding"])
    jax.block_until_ready(list(dev.values()))
    st["res_dev"] = dev
    st["res_src"] = src
    return dev


def make_call_inputs(x, context, x_mask, context_mask):
    """Per-call concat arrays (core = batch), host-side bf16 + transpose."""
    xT = x.astype(NP_BF16).transpose(0, 2, 1)          # [4, 1024, 2048]
    cT = context.astype(NP_BF16).transpose(0, 2, 1)
    xT = np.ascontiguousarray(xT).reshape(N_CORES * DIM, N)
    cT = np.ascontiguousarray(cT).reshape(N_CORES * DIM, M)
    bias = ((context_mask - 1.0) * (-MASK_BIAS)).astype(np.float32)
    bias = np.ascontiguousarray(
        bias.reshape(B, 16, 128).transpose(0, 2, 1)).reshape(B * 128, 16)
    xm = np.ascontiguousarray(
        np.broadcast_to(x_mask[:, None, :], (B, HEADS, N)).astype(np.float32)
    ).reshape(B * HEADS, N)
    return {"xT": xT, "cT": cT, "bias": bias, "xm": xm}


def run_device(st, call_arrays):
    args = [call_arrays[n] for n in st["in_names"]] + st["dev_zeros"]
    out = st["sharded"](*args)
    st["jax"].block_until_ready(out)
    return out


def assemble_output(y_dev, context_mask, bo):
    out = np.asarray(y_dev).astype(np.float32).reshape(B, N, DIM)
    if bo.any():
        out += bo
    for b in range(B):
        if context_mask[b].sum() == 0.0:
            out[b] = bo
    return out


def kernel(x, context, x_mask, context_mask, Wq, Wkv, Wo, bo):
    x = np.asarray(x, dtype=np.float32)
    context = np.asarray(context, dtype=np.float32)
    x_mask = np.asarray(x_mask, dtype=np.float32)
    context_mask = np.asarray(context_mask, dtype=np.float32)
    Wq = np.asarray(Wq, dtype=np.float32)
    Wkv = np.asarray(Wkv, dtype=np.float32)
    Wo = np.asarray(Wo, dtype=np.float32)
    bo = np.asarray(bo, dtype=np.float32)

    st = get_executor()
    res = _resident_weights(st, Wq, Wkv, Wo)
    call_arrays = dict(res)
    call_arrays.update(make_call_inputs(x, context, x_mask, context_mask))
    out = run_device(st, call_arrays)
    return assemble_output(out[0], context_mask, bo)


if __name__ == "__main__":
    rng = np.random.default_rng(0)
    ins = {
        "x": rng.standard_normal((B, N, DIM), dtype=np.float32),
        "context": rng.standard_normal((B, M, DIM), dtype=np.float32),
        "x_mask": (rng.random((B, N)) > 0.1).astype(np.float32),
        "context_mask": (rng.random((B, M)) > 0.1).astype(np.float32),
        "Wq": (rng.standard_normal((DIM, DIM), dtype=np.float32) * 0.02),
        "Wkv": (rng.standard_normal((DIM, 2 * DIM), dtype=np.float32) * 0.02),
        "Wo": (rng.standard_normal((DIM, DIM), dtype=np.float32) * 0.02),
        "bo": np.zeros((DIM,), np.float32),
    }
    out = kernel(**ins)
    print("kernel ran, out shape", out.shape)


# revision 9
# speedup vs baseline: 32.1422x; 32.1422x over previous
"""Trainium2 Bass kernel for nn_CrossAttention (B=4, N=M=2048, DIM=1024, H=16, Dh=64).

Wire-optimized design (the axon tunnel moves ~75 MB/s, so transfer bytes are the
metric that matters):
  - 4 cores, one batch each, all 16 heads per core -> x/context are sent exactly
    once (no duplication), y comes back complete (no host partial sums).
  - x/context cross the wire in bf16, transposed on host; y returns in bf16.
  - Weights are uploaded once and kept device-resident across kernel() calls
    (verified by array_equal on repeat calls); output zero-buffers stay resident.
  - The jitted SPMD callable is built once and cached at module level.

Device program (per core, 16 heads, bf16 matmuls for QKV/S/PV, f32r for the
output projection):
  xT, cT:  [1024, 2048] bf16 (contraction dim on partitions)
  Q^T/K^T: [1024, 2048]  (inner on partitions) -> QK^T contraction over d=64
  S^T:     [m, n] tiles  (keys on partitions)  -> context-mask bias is
           per-partition, folded into the ACT Exp (bias + scale=1/8)
  V':      [m, 65*16]    V plus a ones-column per head -> PV matmul also emits
           softmax denominators (row 64 of the [65, n] psum).
  Normalization: 1/s premultiplied by x_mask (zeroes masked query rows on
  device), broadcast via selector matmul, one DVE multiply on O^T.
"""

import sys
import numpy as np

sys.path.insert(0, "/opt/trn_rl_repo")

import concourse.bass as bass  # noqa: E402
import concourse.tile as tile  # noqa: E402
from concourse import mybir  # noqa: E402
from contextlib import ExitStack  # noqa: E402

F32 = mybir.dt.float32
F32R = mybir.dt.float32r
BF16 = mybir.dt.bfloat16
NP_BF16 = mybir.dt.np(BF16)
EXP = mybir.ActivationFunctionType.Exp
MULT = mybir.AluOpType.mult

B, N, M, DIM = 4, 2048, 2048, 1024
HEADS, DH = 16, 64
N_CORES = 4
MASK_BIAS = -10000.0


def _legalize_waits(nc):
    """This walrus build accepts at most one sync-wait per TPB instruction;
    hoist extra waits onto single-wait NoOps on the same engine queue."""
    ctr = 0

    def fix(bb):
        nonlocal ctr
        new_insts, changed = [], False
        for inst in bb.instructions:
            si = inst.sync_info
            if si is not None and si.on_wait is not None and len(si.on_wait) > 1:
                waits = list(si.on_wait)
                for w in waits[:-1]:
                    ctr += 1
                    new_insts.append(mybir.InstNoOp(
                        name=f"waitnop-{ctr}", engine=inst.engine, ins=[], outs=[],
                        sync_info=mybir.SyncInfo(on_wait=[w], on_update=[]),
                    ))
                inst.sync_info = mybir.SyncInfo(
                    on_wait=[waits[-1]], on_update=list(si.on_update or []))
                changed = True
            new_insts.append(inst)
        if changed:
            bb.instructions.clear()
            for i in new_insts:
                bb.add_instruction(i)

    for fn in nc.m.functions:
        for bb in fn.blocks:
            fix(bb)
    for q in nc.m.queues or []:
        for bb in q.blocks:
            fix(bb)
    return ctr


def build_program():
    nc = bass.Bass()
    # per-call inputs (natural layout; transposed on device via DMA xbar)
    x_d = nc.dram_tensor("xn", [N, DIM], BF16, kind="ExternalInput")
    c_d = nc.dram_tensor("cn", [M, DIM], BF16, kind="ExternalInput")
    bias_d = nc.dram_tensor("bias", [128, 16], F32, kind="ExternalInput")
    xm_d = nc.dram_tensor("xm", [HEADS, N], F32, kind="ExternalInput")
    # resident inputs (weights/constants)
    wq_d = nc.dram_tensor("wq", [DIM, DIM], BF16, kind="ExternalInput")
    wk_d = nc.dram_tensor("wk", [DIM, DIM], BF16, kind="ExternalInput")
    wv_d = nc.dram_tensor("wv", [DIM, DIM], BF16, kind="ExternalInput")
    wo_d = nc.dram_tensor("wo", [DIM, DIM], F32R, kind="ExternalInput")
    sel_d = nc.dram_tensor("sel", [HEADS, DIM], F32R, kind="ExternalInput")
    ones_d = nc.dram_tensor("ones", [128, 16], BF16, kind="ExternalInput")
    y_d = nc.dram_tensor("y", [N, DIM], BF16, kind="ExternalOutput")
    oscr_d = nc.dram_tensor("oscr", [8, 128, N], F32R)  # internal scratch

    KT = DIM // 128  # 8 contraction tiles
    with tile.TileContext(nc) as tc, ExitStack() as ctx:
        persist = ctx.enter_context(tc.tile_pool(name="persist", bufs=1))
        psum = ctx.enter_context(tc.tile_pool(name="psum", bufs=2, space="PSUM"))
        psumO = ctx.enter_context(tc.tile_pool(name="psumO", bufs=4, space="PSUM"))

        bias_sb = persist.tile([128, 16], F32, name="bias_sb")
        sel_sb = persist.tile([HEADS, DIM], F32R, name="sel_sb")
        xm_sb = persist.tile([HEADS, N], F32, name="xm_sb")
        s_sb = persist.tile([HEADS, N], F32, name="s_sb")

        nc.sync.dma_start(out=bias_sb, in_=bias_d[:, :])
        nc.sync.dma_start(out=sel_sb, in_=sel_d[:, :])
        nc.sync.dma_start(out=xm_sb, in_=xm_d[:, :])

        # pool spanning phases A+B (K^T and V' persist through attention)
        ctxAB = ctx.enter_context(ExitStack())
        ab = ctxAB.enter_context(tc.tile_pool(name="ab", bufs=1))
        kT = [ab.tile([128, M], BF16, name=f"kT{pt}") for pt in range(KT)]
        vv = [ab.tile([128, 65 * HEADS], BF16, name=f"vv{mt}") for mt in range(16)]

        # ---------------- Phase A: K^T and V' projections -------------------
        with tc.tile_pool(name="phaseA", bufs=1) as pa:
            cT = [pa.tile([128, M], BF16, name=f"cT{kt}") for kt in range(KT)]
            wk = [pa.tile([128, DIM], BF16, name=f"wk{kt}") for kt in range(KT)]
            wv = [pa.tile([128, DIM], BF16, name=f"wv{kt}") for kt in range(KT)]
            wk_t = wk_d.rearrange("(ko p) c -> ko p c", p=128)
            wv_t = wv_d.rearrange("(ko p) c -> ko p c", p=128)
            for kt in range(KT):
                nc.sync.dma_start(out=cT[kt],
                                  in_=c_d[:, kt * 128:(kt + 1) * 128],
                                  transpose=True)
                nc.sync.dma_start(out=wk[kt], in_=wk_t[kt])
                nc.sync.dma_start(out=wv[kt], in_=wv_t[kt])

            # K^T: [1024 inner, 2048 m]
            for pt in range(KT):
                for t in range(2):
                    ps = psum.tile([128, 1024], F32, name="ps", tag="ps")
                    for kt in range(KT):
                        for sl in range(2):
                            nc.tensor.matmul(
                                ps[:, sl * 512:(sl + 1) * 512],
                                wk[kt][:, pt * 128:(pt + 1) * 128],
                                cT[kt][:, (2 * t + sl) * 512:(2 * t + sl + 1) * 512],
                                start=(kt == 0), stop=(kt == KT - 1))
                    nc.vector.tensor_copy(
                        out=kT[pt][:, t * 1024:(t + 1) * 1024], in_=ps)

            # V': [m, 65 per head] with ones column at 65j+64
            for mt in range(16):
                vvv = vv[mt].rearrange("p (j c) -> p j c", c=65)
                nc.sync.dma_start(out=vvv[:, :, 64], in_=ones_d[:, :])
            for mt in range(16):
                ps = psum.tile([128, 1024], F32, name="ps", tag="ps")
                for sl in range(2):
                    for kt in range(KT):
                        nc.tensor.matmul(
                            ps[:, sl * 512:(sl + 1) * 512],
                            cT[kt][:, mt * 128:(mt + 1) * 128],
                            wv[kt][:, sl * 512:(sl + 1) * 512],
                            start=(kt == 0), stop=(kt == KT - 1))
                for j in range(HEADS):
                    nc.vector.tensor_copy(
                        out=vv[mt][:, 65 * j:65 * j + 64],
                        in_=ps[:, 64 * j: 64 * j + 64])

        # ---------------- Phase B: per head-pair attention -------------------
        ctxB = ctx.enter_context(ExitStack())
        pb = ctxB.enter_context(tc.tile_pool(name="phaseB", bufs=1))
        xT = [pb.tile([128, N], BF16, name=f"xT{kt}") for kt in range(KT)]
        for kt in range(KT):
            nc.sync.dma_start(out=xT[kt],
                              in_=x_d[:, kt * 128:(kt + 1) * 128],
                              transpose=True)

        wqp_pool = ctxB.enter_context(tc.tile_pool(name="wqp", bufs=2))
        qt_pool = ctxB.enter_context(tc.tile_pool(name="qt", bufs=1))
        pt_pool = ctxB.enter_context(tc.tile_pool(name="ptp", bufs=3))
        st_pool = ctxB.enter_context(tc.tile_pool(name="stp", bufs=4))
        ot_pool = ctxB.enter_context(tc.tile_pool(name="otp", bufs=2))

        for p in range(8):
            wqp = wqp_pool.tile([128, KT, 128], BF16, name="wqp", tag="wqp")
            for kt in range(KT):
                nc.sync.dma_start(
                    out=wqp[:, kt, :],
                    in_=wq_d[kt * 128:(kt + 1) * 128, p * 128:(p + 1) * 128])

            # Q^T for this pair: [128 inner, 2048 n]
            qT = qt_pool.tile([128, N], BF16, name="qT", tag="qT")
            for t in range(2):
                ps = psum.tile([128, 1024], F32, name="ps", tag="ps")
                for kt in range(KT):
                    for sl in range(2):
                        nc.tensor.matmul(
                            ps[:, sl * 512:(sl + 1) * 512],
                            wqp[:, kt, :],
                            xT[kt][:, (2 * t + sl) * 512:(2 * t + sl + 1) * 512],
                            start=(kt == 0), stop=(kt == KT - 1))
                nc.vector.tensor_copy(out=qT[:, t * 1024:(t + 1) * 1024], in_=ps)

            oT_p = ot_pool.tile([128, N], F32R, name="oT_p", tag="oT_p")
            for nt2 in range(2):
                psO = [psumO.tile([65, 512], F32, name="psO", tag="psO")
                       for _ in range(4)]
                for mt in range(16):
                    for side in range(2):
                        rows = slice(side * 64, side * 64 + 64)
                        jj = 2 * p + side
                        psS = psum.tile([128, 1024], F32, name="ps", tag="ps")
                        for ncs in range(2):
                            nt_c = nt2 * 1024 + ncs * 512
                            nc.tensor.matmul(
                                psS[:, ncs * 512:(ncs + 1) * 512],
                                kT[p][rows, mt * 128:(mt + 1) * 128],
                                qT[rows, nt_c:nt_c + 512],
                                start=True, stop=True,
                                tile_position=(side * 64, 0))
                        pt_t = pt_pool.tile([128, 1024], BF16, name="pt_t", tag="pt")
                        nc.scalar.activation(
                            out=pt_t, in_=psS, func=EXP,
                            bias=bias_sb[:, mt:mt + 1], scale=0.125)
                        for ncs in range(2):
                            nc.tensor.matmul(
                                psO[side * 2 + ncs],
                                vv[mt][:, 65 * jj:65 * jj + 65],
                                pt_t[:, ncs * 512:(ncs + 1) * 512],
                                start=(mt == 0), stop=(mt == 15))
                for side in range(2):
                    jj = 2 * p + side
                    for ncs in range(2):
                        po = psO[side * 2 + ncs]
                        c0 = nt2 * 1024 + ncs * 512
                        chunk = slice(c0, c0 + 512)
                        if side == 0:
                            nc.vector.tensor_copy(out=oT_p[0:64, chunk], in_=po[0:64, :])
                            st = st_pool.tile([65, 512], F32R, name="st", tag="st")
                            nc.vector.tensor_copy(out=st[64:65, :], in_=po[64:65, :])
                            nc.sync.dma_start(out=s_sb[jj:jj + 1, chunk], in_=st[64:65, :].bitcast(F32))
                        else:
                            st = st_pool.tile([65, 512], F32R, name="st", tag="st")
                            nc.vector.tensor_copy(out=st, in_=po)
                            nc.sync.dma_start(out=oT_p[64:128, chunk], in_=st[0:64, :])
                            nc.sync.dma_start(out=s_sb[jj:jj + 1, chunk], in_=st[64:65, :].bitcast(F32))
            nc.sync.dma_start(out=oscr_d[p], in_=oT_p)
        ctxB.close()
        ctxAB.close()

        # ---------------- Phase C: normalize + output projection -------------
        with tc.tile_pool(name="phaseC", bufs=1) as pc, \
             tc.tile_pool(name="ypool", bufs=2) as ypool:
            oTc = [pc.tile([128, N], F32R, name=f"oTc{pt}") for pt in range(KT)]
            wo_sb = [pc.tile([128, DIM], F32R, name=f"wo{kt}") for kt in range(KT)]
            recip_f = pc.tile([HEADS, N], F32, name="recip_f")
            recip_m = pc.tile([HEADS, N], F32, name="recip_m")
            recip_r = pc.tile([HEADS, N], F32R, name="recip_r")
            for pt in range(KT):
                nc.sync.dma_start(out=oTc[pt], in_=oscr_d[pt])
            wo_t = wo_d.rearrange("(ko p) c -> ko p c", p=128)
            for kt in range(KT):
                nc.sync.dma_start(out=wo_sb[kt], in_=wo_t[kt])
            nc.vector.reciprocal(out=recip_f, in_=s_sb)
            nc.vector.tensor_tensor(out=recip_m, in0=recip_f, in1=xm_sb, op=MULT)
            nc.vector.tensor_copy(out=recip_r, in_=recip_m)

            for pt in range(KT):
                for ncr in range(2):
                    psR = psum.tile([128, 1024], F32, name="ps", tag="ps")
                    for sl in range(2):
                        c0 = (ncr * 2 + sl) * 512
                        nc.tensor.matmul(
                            psR[:, sl * 512:(sl + 1) * 512],
                            sel_sb[:, pt * 128:(pt + 1) * 128],
                            recip_r[:, c0:c0 + 512],
                            start=True, stop=True)
                    nc.vector.tensor_tensor(
                        out=oTc[pt][:, ncr * 1024:(ncr + 1) * 1024],
                        in0=oTc[pt][:, ncr * 1024:(ncr + 1) * 1024],
                        in1=psR, op=MULT)

            for nt in range(16):
                psY = psum.tile([128, 1024], F32, name="ps", tag="ps")
                for half in range(2):
                    for kt in range(KT):
                        nc.tensor.matmul(
                            psY[:, half * 512:(half + 1) * 512],
                            oTc[kt][:, nt * 128:(nt + 1) * 128],
                            wo_sb[kt][:, half * 512:(half + 1) * 512],
                            start=(kt == 0), stop=(kt == KT - 1))
                y_t = ypool.tile([128, DIM], BF16, name="y_t", tag="y_t")
                nc.vector.tensor_copy(out=y_t, in_=psY)
                nc.sync.dma_start(out=y_d[nt * 128:(nt + 1) * 128, :], in_=y_t)

    _legalize_waits(nc)
    return nc


_STATE = {}


def get_program():
    if "nc" not in _STATE:
        _STATE["nc"] = build_program()
    return _STATE["nc"]


def _build_executor():
    import jax
    from jax.sharding import Mesh, PartitionSpec, NamedSharding
    from jax.experimental.shard_map import shard_map
    from concourse import bass2jax

    bass2jax.install_neuronx_cc_hook()
    nc = get_program()

    partition_name = nc.partition_id_tensor.name if nc.partition_id_tensor else None
    in_names, out_names, out_avals, zero_outs = [], [], [], []
    for alloc in nc.m.functions[0].allocations:
        if not isinstance(alloc, mybir.MemoryLocationSet):
            continue
        name = alloc.memorylocations[0].name
        if alloc.kind == "ExternalInput":
            if name != partition_name:
                in_names.append(name)
        elif alloc.kind == "ExternalOutput":
            out_names.append(name)
            shape = tuple(alloc.tensor_shape)
            dtype = mybir.dt.np(alloc.dtype)
            out_avals.append(jax.core.ShapedArray(shape, dtype))
            zero_outs.append(np.zeros(shape, dtype))
    n_params = len(in_names)
    n_outs = len(out_avals)
    all_in_names = list(in_names) + list(out_names)
    if partition_name is not None:
        all_in_names.append(partition_name)

    def _body(*args):
        ins = args[:n_params]
        zeros = list(args[n_params:])
        operands = list(ins) + zeros
        if partition_name is not None:
            operands.append(bass2jax.partition_id_tensor())
        outs = bass2jax._bass_exec_p.bind(
            *operands,
            out_avals=tuple(out_avals),
            in_names=tuple(all_in_names),
            out_names=tuple(out_names),
            lowering_input_output_aliases=(),
            sim_require_finite=False,
            sim_require_nnan=False,
            nc=nc,
        )
        return tuple(outs)

    devices = jax.devices()[:N_CORES]
    mesh = Mesh(np.asarray(devices), ("core",))
    sharded = jax.jit(
        shard_map(_body, mesh=mesh,
                  in_specs=(PartitionSpec("core"),) * (n_params + n_outs),
                  out_specs=(PartitionSpec("core"),) * n_outs,
                  check_rep=False),
        keep_unused=True)
    sharding = NamedSharding(mesh, PartitionSpec("core"))

    dev_zeros = [jax.device_put(
        np.zeros((N_CORES * z.shape[0], *z.shape[1:]), z.dtype), sharding)
        for z in zero_outs]
    jax.block_until_ready(dev_zeros)

    st = {"jax": jax, "sharding": sharding, "sharded": sharded,
          "in_names": in_names, "out_names": out_names,
          "out_avals": out_avals, "dev_zeros": dev_zeros,
          "res_names": ("wq", "wk", "wv", "wo", "sel", "ones"),
          "res_dev": None, "res_src": None}
    return st


def get_executor():
    if "exec" not in _STATE:
        _STATE["exec"] = _build_executor()
    return _STATE["exec"]


def _resident_weights(st, Wq, Wkv, Wo):
    """Upload weights/constants once; reuse device copies while they match."""
    jax = st["jax"]
    src = (Wq, Wkv, Wo)
    if st["res_dev"] is not None:
        old = st["res_src"]
        same = all(o is n for o, n in zip(old, src))
        if not same:
            same = all(np.array_equal(o, n) for o, n in zip(old, src))
        if same:
            return st["res_dev"]
    sel = np.zeros((HEADS, DIM), np.float32)
    for j in range(HEADS):
        sel[j, 64 * j:64 * j + 64] = 1.0
    host = {
        "wq": Wq.astype(NP_BF16),
        "wk": Wkv[:, :DIM].astype(NP_BF16),
        "wv": Wkv[:, DIM:].astype(NP_BF16),
        "wo": Wo.astype(np.float32),
        "sel": sel,
        "ones": np.ones((128, 16), NP_BF16),
    }
    dev = {}
    for name, arr in host.items():
        tiled = np.ascontiguousarray(
            np.broadcast_to(arr[None], (N_CORES, *arr.shape))
        ).reshape(N_CORES * arr.shape[0], *arr.shape[1:])
        dev[name] = jax.device_put(tiled, st["sharding"])
    jax.block_until_ready(list(dev.values()))
    st["res_dev"] = dev
    st["res_src"] = src
    return dev


def make_call_inputs(x, context, x_mask, context_mask):
    """Per-call concat arrays (core = batch), natural layout, bf16 on the wire.
    The device transposes x/context via the DMA xbar."""
    xn = x.astype(NP_BF16).reshape(N_CORES * N, DIM)
    cn = context.astype(NP_BF16).reshape(N_CORES * M, DIM)
    bias = ((context_mask - 1.0) * (-MASK_BIAS)).astype(np.float32)
    bias = np.ascontiguousarray(
        bias.reshape(B, 16, 128).transpose(0, 2, 1)).reshape(B * 128, 16)
    xm = np.ascontiguousarray(
        np.broadcast_to(x_mask[:, None, :], (B, HEADS, N)).astype(np.float32)
    ).reshape(B * HEADS, N)
    return {"xn": xn, "cn": cn, "bias": bias, "xm": xm}


def run_device(st, call_arrays):
    args = [call_arrays[n] for n in st["in_names"]] + st["dev_zeros"]
    out = st["sharded"](*args)
    st["jax"].block_until_ready(out)
    return out


def assemble_output(y_dev, context_mask, bo):
    out = np.asarray(y_dev).astype(np.float32).reshape(B, N, DIM)
    if bo.any():
        out += bo
    for b in range(B):
        if context_mask[b].sum() == 0.0:
            out[b] = bo
    return out


def kernel(x, context, x_mask, context_mask, Wq, Wkv, Wo, bo):
    x = np.asarray(x, dtype=np.float32)
    context = np.asarray(context, dtype=np.float32)
    x_mask = np.asarray(x_mask, dtype=np.float32)
    context_mask = np.asarray(context_mask, dtype=np.float32)
    Wq = np.asarray(Wq, dtype=np.float32)
    Wkv = np.asarray(Wkv, dtype=np.float32)
    Wo = np.asarray(Wo, dtype=np.float32)
    bo = np.asarray(bo, dtype=np.float32)

    st = get_executor()
    jx = None
    res = _resident_weights(st, Wq, Wkv, Wo)
    jx = st["jax"]
    # stage the two big tensors as soon as each cast finishes so the wire
    # transfer overlaps the remaining host work (device_put is async)
    call_arrays = dict(res)
    xn = x.astype(NP_BF16).reshape(N_CORES * N, DIM)
    call_arrays["xn"] = jx.device_put(xn, st["sharding"])
    cn = context.astype(NP_BF16).reshape(N_CORES * M, DIM)
    call_arrays["cn"] = jx.device_put(cn, st["sharding"])
    bias = ((context_mask - 1.0) * (-MASK_BIAS)).astype(np.float32)
    call_arrays["bias"] = np.ascontiguousarray(
        bias.reshape(B, 16, 128).transpose(0, 2, 1)).reshape(B * 128, 16)
    call_arrays["xm"] = np.ascontiguousarray(
        np.broadcast_to(x_mask[:, None, :], (B, HEADS, N)).astype(np.float32)
    ).reshape(B * HEADS, N)
    out = run_device(st, call_arrays)
    return assemble_output(out[0], context_mask, bo)


if __name__ == "__main__":
    rng = np.random.default_rng(0)
    ins = {
        "x": rng.standard_normal((B, N, DIM), dtype=np.float32),
        "context": rng.standard_normal((B, M, DIM), dtype=np.float32),
        "x_mask": (rng.random((B, N)) > 0.1).astype(np.float32),
        "context_mask": (rng.random((B, M)) > 0.1).astype(np.float32),
        "Wq": (rng.standard_normal((DIM, DIM), dtype=np.float32) * 0.02),
        "Wkv": (rng.standard_normal((DIM, 2 * DIM), dtype=np.float32) * 0.02),
        "Wo": (rng.standard_normal((DIM, DIM), dtype=np.float32) * 0.02),
        "bo": np.zeros((DIM,), np.float32),
    }
    out = kernel(**ins)
    print("kernel ran, out shape", out.shape)
